# revision 28
# baseline (speedup 1.0000x reference)
"""HONU (order-2, L=64) forward as a per-row quadratic form on 8 trn2 cores.

Reference: out[i] = sum_{j<=k} W[p(j,k)] x[i,j] x[i,k] + b = x_i^T A x_i + b
with A upper-triangular scattered from W.  Pure data parallel over the batch.

Host preprocessing (fused with the shard split): x is cast to bf16 and laid
out per-core TRANSPOSED and block-packed:
    xt[cb*64 + m, n] = x[cb*1024 + n, m]        ([128, 1024] contiguous)
i.e. features of rows 0..1023 on partitions 0..63 and of rows 1024..2047 on
partitions 64..127 (the blockdiag trick).  bf16 end-to-end was
host-simulated at rel err ~3e-3; the gate is 2e-2.

Per-core device program (~12 instructions):
  * one contiguous 256KB DMA lands xt; a second small DMA lands the
    constants (blockdiag(A,A) and the block-ones reduction matrix).
  * yt = blockdiag(A,A)^T @ xt (2 bf16 matmuls, N=512) -> PSUM
  * z = xt * yt (2 DVE muls, bf16 out)
  * po[cb, n] = sum_feat z = out_row(cb*1024+n) - b (2 matmuls with the
    block-ones matrix)
  * po staged PSUM->SBUF in 4 chunks alternating ScalarE/VectorE, and two
    out DMAs so the first half's writeback overlaps the second half's
    compute.  Bias is added on host during the gather.
"""

import math
from contextlib import ExitStack
from itertools import combinations_with_replacement

import numpy as np

import concourse.bacc as bacc
import concourse.bass as bass
import concourse.tile as tile
from concourse import mybir
from concourse.bass_utils import run_bass_kernel_spmd

L = 64
ORDER = 2
B = 16384
N_CORES = 8
SHARD = B // N_CORES  # 2048
HALF = SHARD // 2  # 1024
NUM_W = math.comb(L + 1 + ORDER - 1, ORDER)  # 2145 (only first 2080 used)

IDX = np.array(list(combinations_with_replacement(range(L), ORDER)), dtype=np.int32)

F32 = mybir.dt.float32
BF16 = mybir.dt.bfloat16

_program_cache = {}


def _build_program(compile: bool = True) -> bass.Bass:
    nc = bacc.Bacc()

    x_in = nc.declare_dram_parameter("x", [128, 8 * 2 * L], BF16, isOutput=False)
    cons_in = nc.declare_dram_parameter("cons", [128, 130], BF16, isOutput=False)
    out_t = nc.declare_dram_parameter("out", [SHARD, 1], F32, isOutput=True)

    # out rows: partition 0 -> rows 0..1023, partition 1 -> rows 1024..2047.
    out_v = out_t[:, :].rearrange("(cb r) one -> cb (r one)", cb=2)

    with ExitStack() as ctx:
        tc = ctx.enter_context(tile.TileContext(nc))
        consts = ctx.enter_context(tc.tile_pool(name="consts", bufs=1))
        xt_pool = ctx.enter_context(tc.tile_pool(name="xt", bufs=1))
        z_pool = ctx.enter_context(tc.tile_pool(name="z", bufs=2))
        out_pool = ctx.enter_context(tc.tile_pool(name="outp", bufs=2))
        ps_yt0 = ctx.enter_context(tc.tile_pool(name="ps_yt0", bufs=1, space="PSUM"))
        ps_yt1 = ctx.enter_context(tc.tile_pool(name="ps_yt1", bufs=1, space="PSUM"))
        ps_o0 = ctx.enter_context(tc.tile_pool(name="ps_o0", bufs=1, space="PSUM"))
        ps_o1 = ctx.enter_context(tc.tile_pool(name="ps_o1", bufs=1, space="PSUM"))

        # consts on the gpsimd (SWDGE) path so it runs concurrently with the
        # x transfer on the sync HWDGE ring (HWDGE DMAs are FIFO per ring).
        cons = consts.tile([128, 130], BF16)
        nc.gpsimd.dma_start(out=cons[:], in_=cons_in[:, :])

        # x in two halves on the sync ring: the first half's completion
        # unblocks MM0 while the second half still streams.
        xt = xt_pool.tile([128, 1024], BF16)
        nc.sync.dma_start(out=xt[:, 0:512], in_=x_in[:, 0:512])
        nc.sync.dma_start(out=xt[:, 512:1024], in_=x_in[:, 512:1024])

        a2 = cons[:, 0:128]
        ew = cons[:, 128:130]

        # yt = blockdiag(A,A)^T @ xt   (bf16, N=512).  Separate PSUM tiles
        # per half: PSUM WAR tracking is tile-coarse, and a shared tile
        # serializes MM1 behind mul0.
        pyt0 = ps_yt0.tile([128, 512], F32)
        pyt1 = ps_yt1.tile([128, 512], F32)
        nc.tensor.matmul(pyt0[:], lhsT=a2, rhs=xt[:, 0:512], start=True, stop=True)
        nc.tensor.matmul(pyt1[:], lhsT=a2, rhs=xt[:, 512:1024], start=True, stop=True)

        # z = xt * yt  (bf16 out; in1 reads PSUM fp32) -- separate tiles per
        # half so the two half-chains share nothing but engines.
        z0 = z_pool.tile([128, 512], BF16)
        z1 = z_pool.tile([128, 512], BF16, name="z1")
        nc.vector.tensor_mul(z0[:], xt[:, 0:512], pyt0[:])
        nc.vector.tensor_mul(z1[:], xt[:, 512:1024], pyt1[:])

        # po[cb, n] = out_row(cb*1024 + n) - b
        po0 = ps_o0.tile([2, 512], F32)
        po1 = ps_o1.tile([2, 512], F32)
        nc.tensor.matmul(po0[:], lhsT=ew, rhs=z0[:], start=True, stop=True)
        nc.tensor.matmul(po1[:], lhsT=ew, rhs=z1[:], start=True, stop=True)

        # PSUM -> SBUF staging: one single-engine op per half (DVE for half 0
        # right after its muls, ACT for half 1), each followed by a writeback
        # DMA on a ring fed by the same engine's chain -- no cross-engine
        # hops between copy and trigger, and the two halves fully overlap.
        out_sb0 = out_pool.tile([2, 512], F32)
        out_sb1 = out_pool.tile([2, 512], F32, name="out_sb1")
        nc.vector.tensor_copy(out_sb0[:], po0[:])
        nc.sync.dma_start(out=out_v[:, 0:512], in_=out_sb0[:])
        nc.scalar.activation(
            out_sb1[:], po1[:], mybir.ActivationFunctionType.Copy
        )
        # ACT ring: same engine as the copy above, so the trigger follows
        # with no cross-engine hop (measured faster than queueing on sync
        # behind out0 despite the ACT ring's slower trigger).
        nc.scalar.dma_start(out=out_v[:, 512:1024], in_=out_sb1[:])

    if compile:
        nc.compile()
    return nc


def _get_program() -> bass.Bass:
    if "nc" not in _program_cache:
        _program_cache["nc"] = _build_program()
    return _program_cache["nc"]


def _host_constants(W: np.ndarray):
    from ml_dtypes import bfloat16

    A = np.zeros((L, L), dtype=np.float32)
    A[IDX[:, 0], IDX[:, 1]] = W[: IDX.shape[0]].astype(np.float32)
    C = np.zeros((128, 130), dtype=np.float32)
    C[:64, 0:64] = A
    C[64:, 64:128] = A
    C[:64, 128] = 1.0
    C[64:, 129] = 1.0
    return C.astype(bfloat16)


def _prep_x(x: np.ndarray):
    """Per-core [128, 1024] bf16 with xt[cb*64+m, n] = x[cb*1024+n, m]."""
    from ml_dtypes import bfloat16

    # [core, cb, n, m] -> [core, cb, m, n]
    xr = x.reshape(N_CORES, 2, HALF, L).transpose(0, 1, 3, 2)
    return np.ascontiguousarray(xr.reshape(N_CORES, 128, HALF)).astype(bfloat16)


def _run(x, W, b, trace=False):
    x = np.ascontiguousarray(np.asarray(x, dtype=np.float32))
    W = np.asarray(W, dtype=np.float32)
    b = np.asarray(b, dtype=np.float32)
    assert x.shape == (B, L), x.shape

    C = _host_constants(W)
    xh = _prep_x(x)
    nc = _get_program()
    in_maps = [{"x": xh[c], "cons": C} for c in range(N_CORES)]
    res = run_bass_kernel_spmd(nc, in_maps, core_ids=list(range(N_CORES)), trace=trace)
    # Device emits po[cb, n] = out_row(cb*1024+n) - b per shard; add bias here.
    dev = np.stack([np.asarray(res.results[c]["out"]) for c in range(N_CORES)])
    out = dev.reshape(B, 1) + b.reshape(-1)[0]
    return np.ascontiguousarray(out, dtype=np.float32), res


def kernel(x, W, b):
    out, _ = _run(x, W, b)
    return out


# revision 31
# speedup vs baseline: 1.0356x; 1.0356x over previous
"""HONU (order-2, L=64) forward as a per-row quadratic form on 8 trn2 cores.

Reference: out[i] = sum_{j<=k} W[p(j,k)] x[i,j] x[i,k] + b = x_i^T A x_i + b
with A upper-triangular scattered from W.  Pure data parallel over the batch.

Host preprocessing (fused with the shard split): x is cast to bf16 and laid
out per-core TRANSPOSED and block-packed:
    xt[cb*64 + m, n] = x[cb*1024 + n, m]        ([128, 1024] contiguous)
i.e. features of rows 0..1023 on partitions 0..63 and of rows 1024..2047 on
partitions 64..127 (the blockdiag trick).  bf16 end-to-end was
host-simulated at rel err ~3e-3; the gate is 2e-2.

Per-core device program (~12 instructions):
  * one contiguous 256KB DMA lands xt; a second small DMA lands the
    constants (blockdiag(A,A) and the block-ones reduction matrix).
  * yt = blockdiag(A,A)^T @ xt (2 bf16 matmuls, N=512) -> PSUM
  * z = xt * yt (2 DVE muls, bf16 out)
  * po[cb, n] = sum_feat z = out_row(cb*1024+n) - b (2 matmuls with the
    block-ones matrix)
  * po staged PSUM->SBUF in 4 chunks alternating ScalarE/VectorE, and two
    out DMAs so the first half's writeback overlaps the second half's
    compute.  Bias is added on host during the gather.
"""

import math
from contextlib import ExitStack
from itertools import combinations_with_replacement

import numpy as np

import concourse.bacc as bacc
import concourse.bass as bass
import concourse.tile as tile
from concourse import mybir
from concourse.bass_utils import run_bass_kernel_spmd

L = 64
ORDER = 2
B = 16384
N_CORES = 8
SHARD = B // N_CORES  # 2048
HALF = SHARD // 2  # 1024
NUM_W = math.comb(L + 1 + ORDER - 1, ORDER)  # 2145 (only first 2080 used)

IDX = np.array(list(combinations_with_replacement(range(L), ORDER)), dtype=np.int32)

F32 = mybir.dt.float32
BF16 = mybir.dt.bfloat16

_program_cache = {}


def _build_program(compile: bool = True) -> bass.Bass:
    nc = bacc.Bacc()

    x_in = nc.declare_dram_parameter("x", [128, 8 * 2 * L], BF16, isOutput=False)
    cons_in = nc.declare_dram_parameter("cons", [128, 130], BF16, isOutput=False)
    out_t = nc.declare_dram_parameter("out", [SHARD, 1], F32, isOutput=True)

    # out rows: partition 0 -> rows 0..1023, partition 1 -> rows 1024..2047.
    out_v = out_t[:, :].rearrange("(cb r) one -> cb (r one)", cb=2)

    with ExitStack() as ctx:
        tc = ctx.enter_context(tile.TileContext(nc))
        consts = ctx.enter_context(tc.tile_pool(name="consts", bufs=1))
        xt_pool = ctx.enter_context(tc.tile_pool(name="xt", bufs=1))
        z_pool = ctx.enter_context(tc.tile_pool(name="z", bufs=2))
        out_pool = ctx.enter_context(tc.tile_pool(name="outp", bufs=2))
        ps_yt0 = ctx.enter_context(tc.tile_pool(name="ps_yt0", bufs=1, space="PSUM"))
        ps_yt1 = ctx.enter_context(tc.tile_pool(name="ps_yt1", bufs=1, space="PSUM"))
        ps_o0 = ctx.enter_context(tc.tile_pool(name="ps_o0", bufs=1, space="PSUM"))
        ps_o1 = ctx.enter_context(tc.tile_pool(name="ps_o1", bufs=1, space="PSUM"))

        # consts on the gpsimd (SWDGE) path so it runs concurrently with the
        # x transfer on the sync HWDGE ring (HWDGE DMAs are FIFO per ring).
        cons = consts.tile([128, 130], BF16)
        nc.gpsimd.dma_start(out=cons[:], in_=cons_in[:, :])

        # x in two halves on the sync ring: the first half's completion
        # unblocks MM0 while the second half still streams.
        xt = xt_pool.tile([128, 1024], BF16)
        nc.sync.dma_start(out=xt[:, 0:512], in_=x_in[:, 0:512])
        nc.sync.dma_start(out=xt[:, 512:1024], in_=x_in[:, 512:1024])

        a2 = cons[:, 0:128]
        ew = cons[:, 128:130]

        # yt = blockdiag(A,A)^T @ xt   (bf16, N=512).  Separate PSUM tiles
        # per half: PSUM WAR tracking is tile-coarse, and a shared tile
        # serializes MM1 behind mul0.
        pyt0 = ps_yt0.tile([128, 512], F32)
        pyt1 = ps_yt1.tile([128, 512], F32)
        nc.tensor.matmul(pyt0[:], lhsT=a2, rhs=xt[:, 0:512], start=True, stop=True)
        nc.tensor.matmul(pyt1[:], lhsT=a2, rhs=xt[:, 512:1024], start=True, stop=True)

        # z = xt * yt  (bf16 out; in1 reads PSUM fp32) -- separate tiles per
        # half so the two half-chains share nothing but engines.
        z0 = z_pool.tile([128, 512], BF16)
        z1 = z_pool.tile([128, 512], BF16, name="z1")
        nc.vector.tensor_mul(z0[:], xt[:, 0:512], pyt0[:])
        nc.vector.tensor_mul(z1[:], xt[:, 512:1024], pyt1[:])

        # po[cb, n] = out_row(cb*1024 + n) - b
        po0 = ps_o0.tile([2, 512], F32)
        po1 = ps_o1.tile([2, 512], F32)
        nc.tensor.matmul(po0[:], lhsT=ew, rhs=z0[:], start=True, stop=True)
        nc.tensor.matmul(po1[:], lhsT=ew, rhs=z1[:], start=True, stop=True)

        # PSUM -> SBUF staging: one single-engine op per half (DVE for half 0
        # right after its muls, ACT for half 1), each followed by a writeback
        # DMA on a ring fed by the same engine's chain -- no cross-engine
        # hops between copy and trigger, and the two halves fully overlap.
        out_sb0 = out_pool.tile([2, 512], F32)
        out_sb1 = out_pool.tile([2, 512], F32, name="out_sb1")
        nc.vector.tensor_copy(out_sb0[:], po0[:])
        nc.sync.dma_start(out=out_v[:, 0:512], in_=out_sb0[:])
        nc.scalar.activation(
            out_sb1[:], po1[:], mybir.ActivationFunctionType.Copy
        )
        # ACT ring: same engine as the copy above, so the trigger follows
        # with no cross-engine hop (measured faster than queueing on sync
        # behind out0 despite the ACT ring's slower trigger).
        nc.scalar.dma_start(out=out_v[:, 512:1024], in_=out_sb1[:])

    if compile:
        nc.compile()
    return nc


def _get_program() -> bass.Bass:
    if "nc" not in _program_cache:
        _program_cache["nc"] = _build_program()
    return _program_cache["nc"]


def _host_constants(W: np.ndarray):
    from ml_dtypes import bfloat16

    A = np.zeros((L, L), dtype=np.float32)
    A[IDX[:, 0], IDX[:, 1]] = W[: IDX.shape[0]].astype(np.float32)
    C = np.zeros((128, 130), dtype=np.float32)
    C[:64, 0:64] = A
    C[64:, 64:128] = A
    C[:64, 128] = 1.0
    C[64:, 129] = 1.0
    return C.astype(bfloat16)


def _prep_x(x: np.ndarray):
    """Per-core [128, 1024] bf16 with xt[cb*64+m, n] = x[cb*1024+n, m]."""
    from ml_dtypes import bfloat16

    # [core, cb, n, m] -> [core, cb, m, n]
    xr = x.reshape(N_CORES, 2, HALF, L).transpose(0, 1, 3, 2)
    return np.ascontiguousarray(xr.reshape(N_CORES, 128, HALF)).astype(bfloat16)


def _run(x, W, b, trace=False):
    x = np.ascontiguousarray(np.asarray(x, dtype=np.float32))
    W = np.asarray(W, dtype=np.float32)
    b = np.asarray(b, dtype=np.float32)
    assert x.shape == (B, L), x.shape

    C = _host_constants(W)
    xh = _prep_x(x)
    nc = _get_program()
    in_maps = [{"x": xh[c], "cons": C} for c in range(N_CORES)]
    res = run_bass_kernel_spmd(nc, in_maps, core_ids=list(range(N_CORES)), trace=trace)
    # Device emits po[cb, n] = out_row(cb*1024+n) - b per shard; add bias here.
    dev = np.stack([np.asarray(res.results[c]["out"]) for c in range(N_CORES)])
    out = dev.reshape(B, 1) + b.reshape(-1)[0]
    return np.ascontiguousarray(out, dtype=np.float32), res


def kernel(x, W, b):
    out, _ = _run(x, W, b)
    return out
